# revision 23
# baseline (speedup 1.0000x reference)
"""Discriminative-loss kernel for Trainium2 (Bass/Tile), 8-core data-parallel.

Layout / algorithm (per core = one batch sample, SPMD over 8 cores):
  inputs per core:  x [d=16, N=262144] fp32 (natural d-major layout),
                    label-derived index tensors (host re-layouts only).
  pass 1:  x --SWDGE-cast--> X_bf bf16 SBUF [128=(16j+dd), M=32768]
           xbar-transpose 128-col blocks -> X_T [n-on-partition, (c,j,17)]
           (col 16 pre-set to ones), then 2048 small accumulating matmuls
           stationary=onehot[128,8] x moving=X_T[128,17] -> PSUM [8,(16+1)]
           = per-cluster sums | counts, 4 PSUM column groups (col-tiled).
  centers: tiny ops: combine groups, divide by counts, transpose,
           replicate -> c_table [128=(16j+dd), 8] fp32.
  pass 2:  ap_gather (GPSIMD) own-center per point -> c_own fp32 ->
           ACT cast bf16 -> DVE diff/square -> matmuls vs blockdiag-ones
           reduce over dd -> s = ||x-c||^2 PSUM [4 slabs x 512] ->
           ACT sqrt -> repack DMA -> e_dense [128, 2048] (chunk-per-core) ->
           relu(d-1) -> square+mask-accumulate per cluster -> V [128, 8].
  host:    centers/dist/reg terms + final mean from [8,17] sums|counts and
           V partials (O(K^2 d) flops on reduced stats only).
"""

import contextlib
import ctypes
import sys
import types

import numpy as np

# ---------------------------------------------------------------------------
# problem constants (hardcoded per contract)
B, D, HH, WW, K = 8, 16, 512, 512, 8
N = HH * WW            # 262144 points per sample
J = 8                  # chunks per core (ap_gather core granularity: 16 parts)
NCORES = 8
DELTA_VAR = 1.0
DELTA_DIST = 2.0

_BF16 = None  # ml_dtypes.bfloat16, resolved lazily


def _bf16():
    global _BF16
    if _BF16 is None:
        import ml_dtypes

        _BF16 = np.dtype(ml_dtypes.bfloat16)
    return _BF16


# ---------------------------------------------------------------------------
# walrus workaround: this toolchain allows only ONE sync-wait per
# instruction; spread extras onto preceding same-engine nops.
def _split_multi_waits(nc):
    from concourse import mybir

    n = 0
    for f in nc.m.functions:
        for bb in f.blocks:
            new_insts = []
            for ins in bb.instructions:
                si = getattr(ins, "sync_info", None)
                waits = list(si.on_wait) if si is not None and si.on_wait else []
                if len(waits) > 1:
                    for w in waits[:-1]:
                        n += 1
                        new_insts.append(
                            mybir.InstNoOp(
                                name=f"I-waitsplit-{n}",
                                engine=ins.engine,
                                bass_nofuse=True,
                                sync_info=mybir.SyncInfo(on_wait=[w], on_update=[]),
                            )
                        )
                    si.on_wait = waits[-1:]
                new_insts.append(ins)
            bb.instructions[:] = new_insts


# ---------------------------------------------------------------------------
# NTFF profiling hook (axon): lets run_bass_kernel_spmd(trace=True) work in
# this container. Harmless if the .so lacks the symbols.
def install_ntff_hook():
    try:
        import antenv

        if "antenv.axon_hooks" in sys.modules:
            return
        so_path = "/opt/axon/libaxon_pjrt.so"
        lib = ctypes.CDLL(so_path)
        if not hasattr(lib, "axon_start_nrt_profile"):
            return
        lib.axon_start_nrt_profile.argtypes = [
            ctypes.POINTER(ctypes.c_int64),
            ctypes.c_size_t,
        ]
        lib.axon_start_nrt_profile.restype = ctypes.c_int64
        lib.axon_stop_nrt_profile.argtypes = [ctypes.c_char_p]
        lib.axon_stop_nrt_profile.restype = ctypes.c_int64

        @contextlib.contextmanager
        def _hook(output_dir, device_ids):
            import jax

            jax.devices()
            if device_ids:
                ids = (ctypes.c_int64 * len(device_ids))(*device_ids)
                rc = lib.axon_start_nrt_profile(ids, len(device_ids))
            else:
                rc = lib.axon_start_nrt_profile(None, 0)
            if rc != 0:
                raise RuntimeError(f"axon_start_nrt_profile rc={rc}")
            try:
                yield
            finally:
                n = lib.axon_stop_nrt_profile(str(output_dir).encode())
                print(f"ntff profile: {n} file(s) -> {output_dir}", file=sys.stderr)

        mod = types.ModuleType("antenv.axon_hooks")
        mod.get_axon_ntff_profile_hook = lambda: _hook
        mod.set_axon_ntff_profile_hook = lambda h: None
        sys.modules["antenv.axon_hooks"] = mod
        antenv.axon_hooks = mod
    except Exception:
        pass


# ---------------------------------------------------------------------------
def build_nc(nt=16, num_devices=NCORES):
    """Build the Bass program.  nt = number of 2048-wide column tiles of the
    per-core X_bf layout (16 for the full problem; smaller for simulation).

    Per-core point count = 8 chunks * M where M = 2048*nt.
    """
    import concourse.bass as bass
    import concourse.tile as tile
    from concourse import mybir

    assert 1 <= nt <= 16
    M = 2048 * nt          # points per chunk
    NPTS = J * M           # points per core
    CB = M // 128          # number of 128-col transpose blocks per chunk-layout
    E_COLS = 2048          # e_dense free size; rows used: 16*j + t (t < nt)

    fp32 = mybir.dt.float32
    bf16 = mybir.dt.bfloat16

    nc = bass.Bass(
        "TRN2", target_bir_lowering=False, debug=False, num_devices=num_devices
    )

    x_in = nc.dram_tensor("x", [D, NPTS], fp32, kind="ExternalInput").ap()
    oh_t = nc.dram_tensor("oh_t", [128, CB, J, K], bf16, kind="ExternalInput").ap()
    # onehot in (j,k)-partition layout: row 8*j + k, col f -> labels[j*M+f]==k
    oh_jk = nc.dram_tensor("oh_jk", [64, M], bf16, kind="ExternalInput").ap()
    lab_e = nc.dram_tensor("lab_e", [128, E_COLS], bf16, kind="ExternalInput").ap()
    # [128, 32] j-selection stationary; cols 8..32 are zero so every matmul
    # writes all 32 partitions of its column group (no stale-PSUM garbage).
    red8 = nc.dram_tensor("red8", [128, 32], bf16, kind="ExternalInput").ap()
    ones128 = nc.dram_tensor("ones128", [128, 1], fp32, kind="ExternalInput").ap()
    out_stats = nc.dram_tensor(
        "out_stats", [K, D + 1], fp32, kind="ExternalOutput"
    ).ap()
    out_var = nc.dram_tensor("out_var", [128, K], fp32, kind="ExternalOutput").ap()

    with tile.TileContext(nc) as tc, contextlib.ExitStack() as ctx:
        # ---------------- pools
        # persistent big tensors
        xbf_pool = ctx.enter_context(tc.tile_pool(name="xbf", bufs=nt))
        xt_pool = ctx.enter_context(tc.tile_pool(name="xt", bufs=min(4, nt)))
        oht_pool = ctx.enter_context(tc.tile_pool(name="oht", bufs=min(4, nt)))
        singles = ctx.enter_context(tc.tile_pool(name="singles", bufs=1))
        tiny = ctx.enter_context(tc.tile_pool(name="tiny", bufs=1))
        p2 = ctx.enter_context(tc.tile_pool(name="p2", bufs=2))
        p2b = ctx.enter_context(tc.tile_pool(name="p2b", bufs=2))
        ps_sums_pool = ctx.enter_context(
            tc.tile_pool(name="ps_sums", bufs=1, space="PSUM")
        )
        ps_e_pool = ctx.enter_context(tc.tile_pool(name="ps_e", bufs=2, space="PSUM"))
        ps_c_pool = ctx.enter_context(tc.tile_pool(name="ps_c", bufs=3, space="PSUM"))

        # ---------------- load constants
        red8_sb = singles.tile([128, 32], bf16)
        nc.sync.dma_start(out=red8_sb[:], in_=red8)
        ones_sb = singles.tile([128, 1], fp32)
        nc.sync.dma_start(out=ones_sb[:], in_=ones128)
        lab_e_sb = singles.tile([128, E_COLS], bf16)
        nc.sync.dma_start(out=lab_e_sb[:], in_=lab_e)

        # ---------------- pass 1: load X (cast to bf16), transpose, cluster sums
        x_r = x_in.rearrange("d (j i) -> j d i", j=J)  # [J, D, M]
        xbf = []
        xt = []
        oht = []
        for t in range(nt):
            xb = xbf_pool.tile([128, 2048], bf16, tag="xbf")
            # partition p = 16*j + dd ; col i local to tile.  out is plain 2-D:
            # src iteration (j, d, i) matches dst (p, i) element order.
            nc.gpsimd.dma_start(
                out=xb[:],
                in_=x_r[:, :, 2048 * t : 2048 * (t + 1)],
            )
            xbf.append(xb)
            # contiguous [128,128] transpose blocks: out[f, p] = in[p, f]
            # with p = 16*j + dd, so chunk j occupies cols 16j..16j+16.
            xtt = xt_pool.tile([128, 16, 128], bf16, tag="xt")
            xt.append(xtt)
            oh = oht_pool.tile([128, 16, J, K], bf16, tag="oht")
            nc.sync.dma_start(out=oh[:], in_=oh_t[:, 16 * t : 16 * (t + 1), :, :])
            oht.append(oh)

        for t in range(nt):
            for cb in range(16):
                # transpose the 128-col block: out rows = points, col groups = (j, dd)
                nc.sync.dma_start_transpose(
                    out=xt[t][:, cb, :],
                    in_=xbf[t][:, 128 * cb : 128 * (cb + 1)],
                )

        # full-bank tile: the CoreSim PSUM pending-zero bookkeeping needs
        # row size == bank size when multiple column groups share a tile
        ps_sums = ps_sums_pool.tile([128, 512], fp32)
        cnt = 0
        for t in range(nt):
            for cb in range(16):
                for j in range(J):
                    g = cnt % 4
                    nc.tensor.matmul(
                        ps_sums[32 * g : 32 * g + K, 0:D],
                        oht[t][:, cb, j, :],
                        xt[t][:, cb, 16 * j : 16 * j + D],
                        start=(cnt < 4),
                        stop=(cnt >= nt * 16 * J - 4),
                        tile_position=(0, 32 * g),
                        skip_group_check=True,
                    )
                    cnt += 1

        # ---------------- centers (tiny ops)
        # TensorTensor may read at most one PSUM operand: copy slabs to SBUF.
        slabs = []
        for g in range(4):
            sl = tiny.tile([K, D], fp32, tag=f"slab{g}")
            nc.scalar.copy(out=sl[:], in_=ps_sums[32 * g : 32 * g + K, 0:D])
            slabs.append(sl)
        s01 = tiny.tile([K, D], fp32, tag="s01")
        nc.vector.tensor_add(s01[:], slabs[0][:], slabs[1][:])
        s23 = tiny.tile([K, D], fp32, tag="s23")
        nc.vector.tensor_add(s23[:], slabs[2][:], slabs[3][:])
        s_sb = tiny.tile([K, D + 1], fp32, tag="s_sb")
        nc.vector.tensor_add(s_sb[:, 0:D], s01[:], s23[:])
        # counts: per-partition masked counts of lab_e, then reduce over
        # partitions with a [128,8]-stationary x ones matmul -> [8, 1]
        cntp = tiny.tile([128, K], fp32, tag="cntp")
        cnt_scratch = singles.tile([128, E_COLS], bf16)
        for k in range(K):
            nc.vector.tensor_scalar(
                out=cnt_scratch[:],
                in0=lab_e_sb[:],
                scalar1=float(k),
                scalar2=None,
                op0=mybir.AluOpType.is_equal,
                op1=mybir.AluOpType.add,
                accum_out=cntp[:, k : k + 1],
            )
        ps_cnt = ps_sums_pool.tile([K, 1], fp32, tag="ps_cnt")
        nc.tensor.matmul(ps_cnt[:], cntp[:], ones_sb[:], start=True, stop=True)
        nc.scalar.copy(out=s_sb[:, D : D + 1], in_=ps_cnt[:])
        nc.sync.dma_start(out=out_stats, in_=s_sb[:])

        rec = tiny.tile([K, 1], fp32, tag="rec")
        nc.vector.reciprocal(rec[:], s_sb[:, D : D + 1])
        c_bf = tiny.tile([K, D], bf16, tag="c_bf")
        nc.vector.tensor_scalar(
            out=c_bf[:],
            in0=s_sb[:, 0:D],
            scalar1=rec[:],
            scalar2=None,
            op0=mybir.AluOpType.mult,
        )
        # W_cblk[(8j+k), (16j'+dd)] = delta(j,j') * c[k, dd]
        w_cblk = singles.tile([64, 128], bf16)
        nc.vector.memset(w_cblk[:], 0.0)
        for j in range(J):
            nc.sync.dma_start(
                out=w_cblk[8 * j : 8 * j + K, 16 * j : 16 * j + D], in_=c_bf[:]
            )

        # ---------------- pass 2
        e_dense = singles.tile([128, E_COLS], bf16)
        nc.vector.memset(e_dense[:], 0.0)
        for t in range(nt):
            ohjk_sb = p2.tile([64, 2048], bf16, tag="ohjk")
            nc.sync.dma_start(
                out=ohjk_sb[:], in_=oh_jk[:, 2048 * t : 2048 * (t + 1)]
            )
            c_ownb = p2.tile([128, 2048], bf16, tag="c_ownb")
            for b in range(4):
                ps_c = ps_c_pool.tile([128, 512], fp32)
                nc.tensor.matmul(
                    ps_c[:],
                    w_cblk[:],
                    ohjk_sb[:, 512 * b : 512 * (b + 1)],
                    start=True,
                    stop=True,
                )
                nc.scalar.copy(
                    out=c_ownb[:, 512 * b : 512 * (b + 1)], in_=ps_c[:]
                )
            dv = p2.tile([128, 2048], bf16, tag="dv")
            nc.vector.tensor_tensor(
                out=dv[:], in0=xbf[t][:], in1=c_ownb[:], op=mybir.AluOpType.subtract
            )
            sq = p2.tile([128, 2048], bf16, tag="sq")
            nc.vector.tensor_tensor(
                out=sq[:], in0=dv[:], in1=dv[:], op=mybir.AluOpType.mult
            )
            ps_e = ps_e_pool.tile([128, 512], fp32)
            for b in range(4):
                nc.tensor.matmul(
                    ps_e[32 * b : 32 * b + 32, :],
                    red8_sb[:],
                    sq[:, 512 * b : 512 * (b + 1)],
                    start=True,
                    stop=True,
                    tile_position=(0, 32 * b),
                    skip_group_check=True,
                )
            s_bf = p2b.tile([128, 512], bf16, tag="s_bf")
            nc.scalar.activation(
                out=s_bf[:], in_=ps_e[:], func=mybir.ActivationFunctionType.Sqrt
            )
            for v in range(4):
                # e_dense[16*t + j, 512*v + f] = s of point (chunk j,
                # pos 2048*t + 512*v + f) -- contiguous partition ranges.
                nc.sync.dma_start(
                    out=e_dense[8 * t : 8 * t + J, 512 * v : 512 * (v + 1)],
                    in_=s_bf[32 * v : 32 * v + J, :],
                )

        m_e = singles.tile([128, E_COLS], bf16)
        nc.vector.tensor_scalar(
            out=m_e[:],
            in0=e_dense[:],
            scalar1=-float(DELTA_VAR),
            scalar2=0.0,
            op0=mybir.AluOpType.add,
            op1=mybir.AluOpType.max,
        )
        msq = singles.tile([128, E_COLS], bf16)
        nc.vector.tensor_tensor(
            out=msq[:], in0=m_e[:], in1=m_e[:], op=mybir.AluOpType.mult
        )
        v_sb = tiny.tile([128, K], fp32, tag="v_sb")
        scratch = singles.tile([128, E_COLS], bf16)
        for k in range(K):
            nc.vector.scalar_tensor_tensor(
                out=scratch[:],
                in0=lab_e_sb[:],
                scalar=float(k),
                in1=msq[:],
                op0=mybir.AluOpType.is_equal,
                op1=mybir.AluOpType.mult,
                accum_out=v_sb[:, k : k + 1],
            )
        nc.sync.dma_start(out=out_var, in_=v_sb[:])

    _split_multi_waits(nc)
    return nc


# ---------------------------------------------------------------------------
# host-side input prep
def prep_core_inputs(x_c, labels_c, nt=16):
    """x_c fp32 [16, NPTS] (contiguous), labels_c int [NPTS] -> in_map."""
    M = 2048 * nt
    NPTS = J * M
    CB = M // 128
    bf = _bf16()
    lab = labels_c.astype(np.int64)

    l3 = lab.reshape(J, CB, 128)  # [j, cb, p]
    oh = (l3[..., None] == np.arange(K)).astype(bf)  # [j, cb, p, k]
    oh_t = np.ascontiguousarray(oh.transpose(2, 1, 0, 3))  # [128, cb, j, k]

    # oh_jk[8*j + k, f] = (labels[j*M + f] == k)
    oh_jk = (
        (lab.reshape(J, 1, M) == np.arange(K).reshape(1, K, 1))
        .reshape(64, M)
        .astype(bf)
    )

    # e_dense layout: partition 8*t + j (t < nt, j < 8),
    #                 col u -> point j*M + 2048*t + u
    lab_e = np.full((128, 2048), -1.0, dtype=np.float32)
    l4 = lab.reshape(J, nt, 2048)  # [j, t, u]
    for t in range(nt):
        lab_e[8 * t : 8 * t + J, :] = l4[:, t, :]
    lab_e = lab_e.astype(bf)

    red8 = np.zeros((128, 32), dtype=bf)
    for p in range(128):
        red8[p, p // 16] = 1.0
    ones128 = np.ones((128, 1), dtype=np.float32)

    return {
        "x": np.ascontiguousarray(x_c),
        "oh_t": oh_t,
        "oh_jk": oh_jk,
        "lab_e": lab_e,
        "red8": red8,
        "ones128": ones128,
    }


def finish_host(stats_list, var_list):
    """Combine per-core [K, D+1] sums|counts and [128, K] var partials."""
    losses = []
    for stats, vparts in zip(stats_list, var_list):
        S = stats[:, :D].astype(np.float64)
        m = stats[:, D].astype(np.float64)
        centers = S / m[:, None]
        V = vparts.astype(np.float64).sum(axis=0)  # [K]
        var_term = np.mean(V / m)
        dif = centers[None, :, :] - centers[:, None, :]
        dmat = np.sqrt((dif**2).sum(-1))
        dmat = dmat + np.eye(K) * DELTA_DIST
        dist_cost = np.clip(DELTA_DIST - dmat, 0.0, None) ** 2
        dist_term = dist_cost.sum() / (K * (K - 1))
        cn = np.sqrt((centers**2).sum(-1))
        reg_term = np.mean(np.clip(cn - np.sqrt(float(D)), 0.0, None) ** 2)
        losses.append(var_term + dist_term + reg_term)
    return np.float32(np.mean(losses))


# ---------------------------------------------------------------------------
_CACHE = {}


def _get_nc():
    if "nc" not in _CACHE:
        _CACHE["nc"] = build_nc(nt=16, num_devices=NCORES)
    return _CACHE["nc"]


def run_device(in_maps, trace=False):
    from concourse.bass_utils import run_bass_kernel_spmd

    if trace:
        install_ntff_hook()
    nc = _get_nc()
    return run_bass_kernel_spmd(
        nc, in_maps, core_ids=list(range(NCORES)), trace=trace
    )


def kernel(data, labels, n_clusters):
    assert int(n_clusters) == K
    assert data.shape == (B, D, HH, WW)
    x = np.asarray(data, dtype=np.float32).reshape(B, D, N)
    lab = np.asarray(labels).reshape(B, N)
    in_maps = [prep_core_inputs(x[c], lab[c]) for c in range(NCORES)]
    res = run_device(in_maps, trace=False)
    stats = [r["out_stats"] for r in res.results]
    vparts = [r["out_var"] for r in res.results]
    return finish_host(stats, vparts)
